# revision 29
# baseline (speedup 1.0000x reference)
"""LSTM cell kernel for Trainium2, 8 NeuronCores, data-parallel over batch.

Math: stacked = x @ Wx + bx + prevh @ Wh
      i,f,o,g = split(stacked, 4, axis=1); i,f,o = sigmoid; g = tanh
      nextc = prevc*f + g*i ; nexth = tanh(nextc)*o

Device strategy (per core, batch shard of 1024 rows):
  - Host pre-concats [x|prevh] and [Wx;Wh] into one K=2048 contraction and
    quantizes both sides to scaled fp8 e4m3 (x*16, W*4096).  Matmuls run in
    MatmulPerfMode.DoubleRow: each instruction contracts 256 k-rows
    (2 x 128 partitions) at 0.5 cycles per output column - 4x the bf16
    row rate under the cost model.
  - Mixed per-gate precision keeps rel-err under the 2e-2 gate: i/f/o use a
    single fp8 pass; the tanh gate g (largest error sensitivity) adds one
    same-scale fp8 residual pass accumulated in PSUM - the weight residual
    x8@RW8 for blocks j0/j1 (whose schedule cannot wait for the rx stream)
    and the activation residual rx8@W8 for j2-j6.  The final block's g is
    single-pass: its epilogue chain is the program's serial tail, so the
    saved matmuls come straight off the wall clock.
  - DMA instruction count is minimized (the HWDGE device serializes at
    ~630ns/DMA): x8/rx8/prevc are pre-chunked to [128, chunk, col] on host
    and stream in a few large DMAs ordered by first-use deadline; each
    state block's weight tiles arrive in one superblock DMA.
  - Per state block (device gate order i,f,g,o): each gate accumulates into
    one [128,1024] 2-bank PSUM tile and evicts in a single full-width
    fused activation (sigmoid/tanh with the 1/65536 scale and the bias) -
    eviction-first, so the in-order ACT queue frees PSUM banks promptly and
    the engine's fixed per-op cost is amortized.  The c/h elementwise chain
    runs on DVE in batch halves and overlaps the next block's matmuls.
    j0 interleaves all four gates k-wise to match the startup x-stream
    rate; the last block runs g,f,i first and pipelines its o-gate epilogue
    half-major so the post-last-matmul serial chain is short.
  - prevc loads and nexth/nextc stores are bf16 (negligible error, halves
    DMA traffic).  Outputs un-transposed and upcast on host.
"""

import os
import sys

sys.path.insert(0, "/opt/trn_rl_repo")
# v2 ASAP tile scheduler: measurably tighter schedule than the legacy flow
os.environ.setdefault("TILE_SCHEDULER", "asap")

import numpy as np

BATCH = 8192
DIM = 1024  # INPUT_DIM == STATE_DIM
K = 2 * DIM  # stacked contraction [x|prevh]
NCORES = 8
B_LOC = BATCH // NCORES  # 1024
N_KT = K // 128  # 16 k-tiles of 128
N_K2 = K // 256  # 8 DoubleRow k-steps of 256
N_GT = 4 * DIM // 128  # 32 gate-block tiles
N_J = DIM // 128  # 8 state blocks

SX = 16.0  # fp8 scale for activations
SW = 4096.0  # fp8 scale for weights
ISCALE = 1.0 / (SX * SW)

_CACHED = {}


def _build_program():
    import ml_dtypes  # noqa: F401
    from concourse import bass, tile
    from concourse.bass import mybir

    f8 = mybir.dt.float8e4
    bf16 = mybir.dt.bfloat16
    f32 = mybir.dt.float32
    AF = mybir.ActivationFunctionType
    DR = mybir.MatmulPerfMode.DoubleRow

    nc = bass.Bass("TRN2", target_bir_lowering=False)
    # activations pre-chunked on host: [partition, kt, col]
    x8_d = nc.dram_tensor("x8", [128, N_KT, B_LOC], f8, kind="ExternalInput")
    rx8_d = nc.dram_tensor("rx8", [128, N_KT, B_LOC], f8, kind="ExternalInput")
    # per-state-block weight superblock: s = 0..3 gates (i,f,g,o), 4 = rw(g)
    w8_d = nc.dram_tensor(
        "w8", [N_J, 128, 5, N_K2, 2, 128], f8, kind="ExternalInput"
    )
    bias_d = nc.dram_tensor("bias", [128, N_GT], f32, kind="ExternalInput")
    pc_d = nc.dram_tensor("pc8", [128, N_J, B_LOC], bf16, kind="ExternalInput")
    hT_d = nc.dram_tensor("hT", [DIM, B_LOC], bf16, kind="ExternalOutput")
    cT_d = nc.dram_tensor("cT", [DIM, B_LOC], bf16, kind="ExternalOutput")

    with tile.TileContext(nc) as tc:
        with (
            tc.tile_pool(name="const", bufs=1) as const_pool,
            tc.tile_pool(name="wp", bufs=3) as w_pool,
            tc.tile_pool(name="gates", bufs=10) as g_pool,
            tc.tile_pool(name="outs", bufs=3) as out_pool,
            tc.tile_pool(name="psum", bufs=4, space="PSUM") as psum_pool,
        ):
            # resident activations: x8 + residual, [128, kt, 1024] fp8,
            # 16KB/partition each.  A DoubleRow moving slice is
            # [:, 2*k2:2*k2+2, h*512:...] -> AP [128, 2, 512].
            xh8_sb = const_pool.tile([128, N_KT, B_LOC], f8)
            rx8_sb = const_pool.tile([128, N_KT, B_LOC], f8)
            pc_all = const_pool.tile([128, N_J, B_LOC], bf16)
            bias_sb = const_pool.tile([128, N_GT], f32)

            # dummy matmuls first in PE program order: warm the PE HAM clock
            # gate (3us busy window) while the startup DMAs stream
            warm_sb = const_pool.tile([1, 128], f8)
            nc.gpsimd.memset(warm_sb[:], 0.0)
            warm_ps = psum_pool.tile([128, 512], f32, tag="ps")
            for _ in range(40):
                nc.tensor.matmul(
                    warm_ps[:, 0:64],
                    warm_sb[:, 0:128],
                    warm_sb[:, 0:64],
                    start=True,
                    stop=True,
                )

            w_tiles = {}

            def load_w(j):
                # blocks j>=1 need only the 4 gate tiles (their g residual
                # pass uses rx, not rw)
                w_sb = w_pool.tile([128, 4, N_K2, 2, 128], f8, tag="w")
                nc.sync.dma_start(w_sb[:], w8_d[j][:, 0:4])
                w_tiles[j] = w_sb

            # startup: j0/j1 weights and the x stream on sync, ordered by
            # first-use deadline (the DMA device is a serialized resource);
            # rx follows - it is first needed by j2's g residual pass
            w0_sb = const_pool.tile([128, 5, N_K2, 2, 128], f8)
            w_tiles[0] = w0_sb
            w1_sb = const_pool.tile([128, 5, N_K2, 2, 128], f8)
            w_tiles[1] = w1_sb
            nc.sync.dma_start(w0_sb[:, 0:1], w8_d[0][:, 0:1])
            nc.sync.dma_start(xh8_sb[:, 0:2], x8_d[:, 0:2])
            nc.sync.dma_start(w0_sb[:, 1:2], w8_d[0][:, 1:2])
            nc.sync.dma_start(xh8_sb[:, 2:4], x8_d[:, 2:4])
            nc.scalar.dma_start(bias_sb[:], bias_d[:])
            nc.sync.dma_start(w0_sb[:, 2:3], w8_d[0][:, 2:3])
            nc.sync.dma_start(xh8_sb[:, 4:8], x8_d[:, 4:8])
            nc.sync.dma_start(w0_sb[:, 3:5], w8_d[0][:, 3:5])
            nc.sync.dma_start(xh8_sb[:, 8:12], x8_d[:, 8:12])
            nc.sync.dma_start(xh8_sb[:, 12:16], x8_d[:, 12:16])
            nc.sync.dma_start(w1_sb[:, 0:2], w8_d[1][:, 0:2])
            nc.sync.dma_start(w1_sb[:, 2:5], w8_d[1][:, 2:5])
            nc.sync.dma_start(pc_all[:, 0:2], pc_d[:, 0:2])
            load_w(2)
            for q in range(4):
                nc.sync.dma_start(
                    rx8_sb[:, 4 * q : 4 * q + 4], rx8_d[:, 4 * q : 4 * q + 4]
                )
            nc.sync.dma_start(pc_all[:, 2:8], pc_d[:, 2:8])

            for j in range(N_J):
                last_j = j == N_J - 1
                # prefetch next block's weight superblock; load this block's
                # prevc early (no deps, so it never parks the ACT queue)
                if j >= 2 and not last_j:
                    load_w(j + 1)
                pc_sb = pc_all[:, j]
                w_all = w_tiles.pop(j)

                def gate_passes(gate):
                    """(stationary s-index, moving tile) per accumulation
                    pass: the g gate adds one same-scale residual pass (the
                    weight residual for j0 - rx arrives late in the startup
                    stream - and the activation residual elsewhere)."""
                    if gate == 2:
                        if j <= 1:
                            return [(2, xh8_sb), (4, xh8_sb)]
                        if j == N_J - 1:
                            # single pass: shortens the final block, whose
                            # epilogue chain is the program's serial tail
                            return [(2, xh8_sb)]
                        return [(2, xh8_sb), (2, rx8_sb)]
                    return [(gate, xh8_sb)]

                def run_gates(gates, evict=True, half_major=False, delay=None):
                    """Issue DoubleRow matmuls for one or more gates,
                    k-interleaved when len>1, then evict eviction-first so
                    the in-order ACT queue frees PSUM banks promptly.  With
                    half_major, a single gate emits all batch-half-0 matmuls
                    before half 1 so ps0 stops (and can evict) early."""
                    tiles = {}
                    steps = {}
                    for gate in gates:
                        psf = psum_pool.tile([128, B_LOC], f32, tag="ps")
                        tiles[gate] = psf
                        vk = [None] * (delay or {}).get(gate, 0)
                        for s, m_sb in gate_passes(gate):
                            for k2 in range(N_K2):
                                vk.append((s, m_sb, k2))
                        steps[gate] = vk
                    n_steps = max(len(s) for s in steps.values())
                    if half_major:
                        (gate,) = gates
                        vk = steps[gate]
                        psf = tiles[gate]
                        for cols in (slice(0, 512), slice(512, B_LOC)):
                            for vi, (s, m_sb, k2) in enumerate(vk):
                                nc.tensor.matmul(
                                    psf[:, cols],
                                    w_all[:, s, k2],
                                    m_sb[:, 2 * k2 : 2 * k2 + 2, cols],
                                    start=vi == 0,
                                    stop=vi == len(vk) - 1,
                                    perf_mode=DR,
                                )
                    else:
                        for vi in range(n_steps):
                            for gate in gates:
                                vk = steps[gate]
                                if vi >= len(vk) or vk[vi] is None:
                                    continue
                                s, m_sb, k2 = vk[vi]
                                first = vi == (delay or {}).get(gate, 0)
                                last = vi == len(vk) - 1
                                psf = tiles[gate]
                                lhsT = w_all[:, s, k2]
                                nc.tensor.matmul(
                                    psf[:, 0:512],
                                    lhsT,
                                    m_sb[:, 2 * k2 : 2 * k2 + 2, 0:512],
                                    start=first,
                                    stop=last,
                                    perf_mode=DR,
                                )
                                nc.tensor.matmul(
                                    psf[:, 512:B_LOC],
                                    lhsT,
                                    m_sb[:, 2 * k2 : 2 * k2 + 2, 512:B_LOC],
                                    start=first,
                                    stop=last,
                                    perf_mode=DR,
                                )
                    if not evict:
                        return tiles
                    for gate in gates:
                        gt = j * 4 + gate
                        psf = tiles[gate]
                        g_sb = g_pool.tile([128, B_LOC], f32, tag="g")
                        func = AF.Tanh if gate == 2 else AF.Sigmoid
                        nc.scalar.activation(
                            g_sb[:],
                            psf[:],
                            func,
                            bias=bias_sb[:, gt : gt + 1],
                            scale=ISCALE,
                        )
                        done[gate] = g_sb
                    return tiles

                done = {}
                if j == 0:
                    # single 4-way interleaved group: PE consumption matches
                    # the startup x-stream arrival rate; o joins 4 rounds
                    # late so its parked matmuls don't clog the depth-4 PE
                    # queue before the o/rw weight slice lands
                    run_gates((0, 1, 2, 3), delay={3: 4})
                elif last_j:
                    # f before i: the last gate's eviction feeds tmp=i*g while
                    # c0=f*pc is already computed
                    for gates in ((2,), (1,), (0,)):
                        run_gates(gates)
                else:
                    for gates in ((0,), (1,), (2,), (3,)):
                        run_gates(gates)

                i_t, f_t, g_t = done[0], done[1], done[2]
                if not last_j:
                    o_t = done[3]
                    # c/h chain in halves: runs on DVE/ACT while the next
                    # block's matmuls occupy the PE; halving shortens the
                    # in-order ACT queue's head-of-line latency
                    c16 = out_pool.tile([128, B_LOC], bf16, tag="c")
                    tmp = out_pool.tile([128, B_LOC], f32, tag="tmp")
                    c0 = out_pool.tile([128, B_LOC], f32, tag="c0")
                    h_sb = out_pool.tile([128, B_LOC], f32, tag="h")
                    h16 = g_pool.tile([128, B_LOC], bf16, tag="h16")
                    for hb in range(2):
                        hs = slice(hb * 512, (hb + 1) * 512)
                        nc.vector.tensor_mul(
                            out=tmp[:, hs], in0=i_t[:, hs], in1=g_t[:, hs]
                        )
                        nc.vector.tensor_mul(
                            out=c0[:, hs], in0=f_t[:, hs], in1=pc_sb[:, hs]
                        )
                        nc.vector.tensor_add(
                            out=c16[:, hs], in0=c0[:, hs], in1=tmp[:, hs]
                        )
                    nc.sync.dma_start(cT_d[j * 128 : (j + 1) * 128, :], c16[:])
                    for hb in range(2):
                        hs = slice(hb * 512, (hb + 1) * 512)
                        nc.scalar.activation(h_sb[:, hs], c16[:, hs], AF.Tanh)
                    for hb in range(2):
                        hs = slice(hb * 512, (hb + 1) * 512)
                        nc.vector.tensor_mul(
                            out=h16[:, hs], in0=h_sb[:, hs], in1=o_t[:, hs]
                        )
                    nc.sync.dma_start(hT_d[j * 128 : (j + 1) * 128, :], h16[:])
                else:
                    # final state block: o matmuls run half-major while the c
                    # chain computes; the epilogue pipelines in batch halves
                    # so the post-last-matmul serial chain is short
                    tiles = run_gates((3,), evict=False, half_major=True)
                    psf_o = tiles[3]
                    gt = j * 4 + 3
                    c16 = out_pool.tile([128, B_LOC], bf16, tag="c")
                    tmp = out_pool.tile([128, B_LOC], f32, tag="tmp")
                    c0 = out_pool.tile([128, B_LOC], f32, tag="c0")
                    h_sb = out_pool.tile([128, B_LOC], f32, tag="h")
                    # dedicated tile: a pool slot here would chain the o
                    # eviction behind unrelated earlier readers
                    o_sb = const_pool.tile([128, B_LOC], f32)
                    h16 = g_pool.tile([128, B_LOC], bf16, tag="h16")
                    for hb in range(2):
                        hs = slice(hb * 512, (hb + 1) * 512)
                        nc.scalar.activation(
                            o_sb[:, hs],
                            psf_o[:, hs],
                            AF.Sigmoid,
                            bias=bias_sb[:, gt : gt + 1],
                            scale=ISCALE,
                        )
                    for hb in range(2):
                        hs = slice(hb * 512, (hb + 1) * 512)
                        nc.vector.tensor_mul(
                            out=c0[:, hs], in0=f_t[:, hs], in1=pc_sb[:, hs]
                        )
                    for hb in range(2):
                        hs = slice(hb * 512, (hb + 1) * 512)
                        nc.vector.tensor_mul(
                            out=tmp[:, hs], in0=i_t[:, hs], in1=g_t[:, hs]
                        )
                        nc.vector.tensor_add(
                            out=c16[:, hs], in0=c0[:, hs], in1=tmp[:, hs]
                        )
                        nc.sync.dma_start(
                            cT_d[j * 128 : (j + 1) * 128, hs], c16[:, hs]
                        )
                    for hb in range(2):
                        hs = slice(hb * 512, (hb + 1) * 512)
                        nc.scalar.activation(h_sb[:, hs], c16[:, hs], AF.Tanh)
                        nc.vector.tensor_mul(
                            out=h16[:, hs], in0=o_sb[:, hs], in1=h_sb[:, hs]
                        )
                        nc.sync.dma_start(
                            hT_d[j * 128 : (j + 1) * 128, hs], h16[:, hs]
                        )

    nc.finalize()
    _install_wait_splitter(nc)
    return nc


def _split_multiwaits(mod: dict) -> dict:
    """This container's walrus encodes at most ONE sync wait per instruction
    (setupSyncWait raises 'Too many sync wait commands'), while Tile emits
    several. Move excess waits onto standalone single-wait EventSemaphore
    instructions inserted just before, on the same engine. All excess waits
    must be monotone (sem-ge-imm) for the serialization to be equivalent.
    """
    for fn in mod.get("functions", []):
        for blk in fn.get("blocks", []):
            insts = blk.get("instructions") or []
            out = []
            for inst in insts:
                si = inst.get("sync_info")
                waits = (si or {}).get("on_wait") or []
                if len(waits) > 1:
                    keep, extra = [], []
                    # keep non-monotone waits (if any) on the instruction
                    for w in waits:
                        (extra if w.get("wait_mode") == "sem-ge-imm" else keep).append(w)
                    if not keep:
                        keep.append(extra.pop())
                    for n, w in enumerate(extra):
                        out.append(
                            {
                                "name": f"{inst['name']}_sw{n}",
                                "opcode": "EventSemaphore",
                                "engine": inst["engine"],
                                "debug": inst.get("debug", 0),
                                "sync_info": {"on_wait": [w], "on_update": []},
                            }
                        )
                    si["on_wait"] = keep
                out.append(inst)
            blk["instructions"] = out
    return mod


def _install_wait_splitter(nc):
    import json as _json

    orig = nc.to_json_bytes

    def patched():
        mod = _json.loads(orig())
        return _json.dumps(_split_multiwaits(mod)).encode()

    nc.to_json_bytes = patched


def _quant(a, scale):
    """Scaled e4m3 value + same-scale residual (both as fp8)."""
    import ml_dtypes

    e4 = ml_dtypes.float8_e4m3
    s = np.clip(a * scale, -240.0, 240.0).astype(np.float32)
    q = s.astype(e4)
    r = np.clip(s - q.astype(np.float32), -240.0, 240.0).astype(e4)
    return q, r


def _prep_shared(Wx, bx, Wh):
    W = np.concatenate([Wx, Wh], axis=0)  # [K, 4*DIM]
    # columns gate*DIM + j*128 + c -> (j*4 + pos)*128 + c with device gate
    # order (i, f, g, o) within each state block j
    W_re = (
        W.reshape(K, 4, N_J, 128)[:, [0, 1, 3, 2]]
        .transpose(0, 2, 1, 3)
        .reshape(K, 4 * DIM)
    )
    W8, RW8 = _quant(W_re, SW)

    def dev_layout(A):  # [K, 4*DIM] fp8 -> [N_GT=(j,gate), 128, N_K2, 2, 128]
        return A.reshape(N_K2, 2, 128, N_GT, 128).transpose(3, 2, 0, 1, 4)

    w4 = dev_layout(W8).reshape(N_J, 4, 128, N_K2, 2, 128)
    rw1 = dev_layout(RW8).reshape(N_J, 4, 128, N_K2, 2, 128)[:, 2:3]
    # superblock: [j, 128, s(4 gates + rw), k2, 2, 128]
    w8_dev = np.ascontiguousarray(
        np.concatenate([w4, rw1], axis=1).transpose(0, 2, 1, 3, 4, 5)
    )
    b_re = bx.reshape(4, N_J, 128)[[0, 1, 3, 2]].transpose(1, 0, 2).reshape(4 * DIM)
    bias_dev = np.ascontiguousarray(b_re.reshape(N_GT, 128).T, dtype=np.float32)
    return w8_dev, bias_dev


def kernel(x, prevh, prevc, Wx, bx, Wh):
    import ml_dtypes
    from concourse import bass_utils

    bf16 = ml_dtypes.bfloat16
    x, prevh, prevc, Wx, bx, Wh = (
        np.asarray(a, dtype=np.float32) for a in (x, prevh, prevc, Wx, bx, Wh)
    )

    if "nc" not in _CACHED:
        _CACHED["nc"] = _build_program()
    nc = _CACHED["nc"]

    w8_dev, bias_dev = _prep_shared(Wx, bx, Wh)

    in_maps = []
    for c in range(NCORES):
        rows = slice(c * B_LOC, (c + 1) * B_LOC)
        xh = np.concatenate([x[rows], prevh[rows]], axis=1)  # [B_LOC, K]
        x8, rx8 = _quant(np.ascontiguousarray(xh.T), SX)  # [K, B_LOC]
        # pre-chunk to [partition, kt, col]
        x8 = np.ascontiguousarray(x8.reshape(N_KT, 128, B_LOC).transpose(1, 0, 2))
        rx8 = np.ascontiguousarray(rx8.reshape(N_KT, 128, B_LOC).transpose(1, 0, 2))
        pc8 = np.ascontiguousarray(
            prevc[rows].T.astype(bf16).reshape(N_J, 128, B_LOC).transpose(1, 0, 2)
        )
        in_maps.append(
            {"x8": x8, "rx8": rx8, "w8": w8_dev, "bias": bias_dev, "pc8": pc8}
        )
    _CACHED["in_maps"] = in_maps

    res = bass_utils.run_bass_kernel_spmd(nc, in_maps, core_ids=list(range(NCORES)))

    nexth = np.empty((BATCH, DIM), np.float32)
    nextc = np.empty((BATCH, DIM), np.float32)
    for c in range(NCORES):
        rows = slice(c * B_LOC, (c + 1) * B_LOC)
        nexth[rows] = np.asarray(res.results[c]["hT"]).astype(np.float32).T
        nextc[rows] = np.asarray(res.results[c]["cT"]).astype(np.float32).T
    return nexth, nextc


if __name__ == "__main__":
    rng = np.random.default_rng(0)
    inputs = {
        "x": rng.standard_normal((BATCH, DIM), np.float32),
        "prevh": rng.standard_normal((BATCH, DIM), np.float32),
        "prevc": rng.standard_normal((BATCH, DIM), np.float32),
        "Wx": (rng.random((DIM, 4 * DIM), np.float32) - 0.5) / 16,
        "bx": (rng.random(4 * DIM, np.float32) - 0.5) / 16,
        "Wh": (rng.random((DIM, 4 * DIM), np.float32) - 0.5) / 16,
    }
    h, c = kernel(**inputs)
    print("ok", h.shape, c.shape, h.dtype)
